# revision 47
# baseline (speedup 1.0000x reference)
"""Causal multi-head attention on 8 Trainium2 NeuronCores — v3 (fp8 DoubleRow).

Sharding: core c -> (batch g = c // 4, head-group p = c % 4, heads 4p..4p+3).

v3 moves the Q/K/V projections and the QK^T scores to fp8e4m3 DoubleRow
matmuls (0.5 PE cycles per moving column vs 1.0 for bf16):

- Inputs arrive from host as fp8 hi/lo pairs (x8 = fp8(x), xr8 = fp8(x - x8))
  laid out [128, hl, blk, kcp, t, 512] so each DoubleRow matmul contracts
  2x128 input features per instruction. Projections compute the 3-term
  correction x8@W8 + xr8@W8 + x8@W8r (error ~0.1%), 12 DR matmuls per
  [128, 512] output tile vs 16 bf16 matmuls.
- Weights are host-scaled by 32 (w' = 32w) so fp8 quantization stays clear
  of the subnormal floor; the 32^2 factor on scores is folded into the exp
  scale (2^-13), and the 32 on V is cancelled by a 32.0 ones-column that
  also produces the softmax denominators.
- Q/K land directly in fp8 [128, pt, 2, S] with a zeroed DoubleRow sub-plane
  (the dk=64 contraction is packed as 64 real + 64 zero rows), so scores run
  DoubleRow at 0.5 cyc/col with no extra layout work. AV + O-proj stay bf16
  (probs/ctx quantization would blow the 2e-2 error budget).

Scheduling: projections stream through 2 PSUM banks and overlap the early
score/exp waves (Act starts ~10us in); AV opens after proj+V release their
banks (~27us) and catches up from the 19-deep probs buffer. Output is
ReduceScattered per 512-row wave (4 collectives) so only the last wave's
RS is exposed at the tail. PE warm-up dummies hold the pstate ramp.
"""

import numpy as np

B, S, D, H = 2, 2048, 1024, 16
DK = D // H
N_CORES = 8
FPC = 256  # features (head dims) per core
SCL = 32.0  # host-side weight scale

_CACHE = {}
MM_LABELS = []  # diagnostic: label of each emitted matmul, in emission order
_LBL = ["?"]


def _build_nc():
    import concourse.mybir as mybir
    import concourse.tile as tile
    from concourse import bacc

    F32 = mybir.dt.float32
    BF16 = mybir.dt.bfloat16
    FP8 = mybir.dt.float8e4
    Exp = mybir.ActivationFunctionType.Exp
    DR = mybir.MatmulPerfMode.DoubleRow

    nc = bacc.Bacc("TRN2", target_bir_lowering=False, debug=False, num_devices=8)
    import os as _os
    MASK_ENG = nc.vector if not _os.environ.get("K_MASK_GPSIMD") else nc.gpsimd

    MM_LABELS.clear()
    _orig_mm = nc.tensor.matmul

    def _mm(*a, **kw):
        MM_LABELS.append(_LBL[0])
        return _orig_mm(*a, **kw)

    nc.tensor.matmul = _mm

    # weights: flat rows of [hl, kcp, t, feat] fp8 (hi/lo), value = fp8(32*w.T)
    # (flat innermost dim keeps DMA descriptors at 4KB, off the
    # min-transfer-time floor)
    cWQ = nc.dram_tensor("cWQ", [128, 4096], FP8, kind="ExternalInput")
    cWK = nc.dram_tensor("cWK", [128, 4096], FP8, kind="ExternalInput")
    cWV = nc.dram_tensor("cWV", [128, 4096], FP8, kind="ExternalInput")
    cWO = nc.dram_tensor("cWO", [128, 2, 1024], BF16, kind="ExternalInput")
    cMASK = nc.dram_tensor("cMASK", [128, 512], BF16, kind="ExternalInput")
    cF = nc.dram_tensor("cF", [128, 260], F32, kind="ExternalInput")
    # activations: flat per-block rows of [hl, kcp, t, c] fp8
    # (x[256*kcp+128*t+p, 512*blk+c]); one 8KB/partition DMA per block
    xq8 = nc.dram_tensor("xq8", [128, 4, 8192], FP8, kind="ExternalInput")
    xk8 = nc.dram_tensor("xk8", [128, 4, 8192], FP8, kind="ExternalInput")
    xv8 = nc.dram_tensor("xv8", [128, 4, 8192], FP8, kind="ExternalInput")
    out = nc.dram_tensor("out", [512, D], BF16, kind="ExternalOutput")

    with tile.TileContext(nc) as tc:
        with (
            tc.tile_pool(name="consts", bufs=1) as consts,
            tc.tile_pool(name="persist", bufs=1) as persist,
            tc.tile_pool(name="xin", bufs=3) as xin,
            tc.tile_pool(name="prp", bufs=1) as prp,
            tc.tile_pool(name="small", bufs=3) as small,
            tc.tile_pool(name="oout", bufs=6) as oout,
            tc.tile_pool(name="dram", bufs=1, space="DRAM") as dram,
        ):
            # ---------------- constants ----------------
            cWQ_s = consts.tile([128, 2, 4, 2, 256], FP8, tag="cWQ", name="cWQ_s")
            cWK_s = consts.tile([128, 2, 4, 2, 256], FP8, tag="cWK", name="cWK_s")
            cWV_s = consts.tile([128, 2, 4, 2, 256], FP8, tag="cWV", name="cWV_s")
            cWO_s = consts.tile([128, 2, 1024], BF16, tag="cWO", name="cWO_s")
            mask4_s = consts.tile([128, 4, 128], BF16, tag="mask", name="mask4_s")
            cF_s = consts.tile([128, 260], F32, tag="cF", name="cF_s")
            warm_s = consts.tile([128, 512], BF16, tag="warm", name="warm_s")
            bq_s = cF_s[:, 0:2]
            bk_s = cF_s[:, 2:4]
            bvt_s = cF_s[:, 4:260].rearrange("p (h x) -> p h x", h=4)

            # ---------------- persistent activations ----------------
            # q/k in fp8, [p, pt, j, col]: j=0 data, j=1 zeros (DoubleRow pad)
            qT8 = persist.tile([128, 2, 2, S], FP8, tag="qT8", name="qT8")
            kT8 = persist.tile([128, 2, 2, S], FP8, tag="kT8", name="kT8")
            xv8_s = persist.tile([128, 4, 2, 4, 2, 512], FP8, tag="xv8", name="xv8_s")
            v_s = persist.tile([128, 16, 4, 65], BF16, tag="v", name="v_s")
            ctxT_s = persist.tile([128, 2, S], BF16, tag="ctxT", name="ctxT_s")

            rs_in = [dram.tile([512, D], BF16, name=f"rs_in{w}") for w in range(4)]
            rs_out = [dram.tile([128, D], BF16, name=f"rs_out{w}") for w in range(4)]

            # ---------------- t=0 setup ----------------
            # softmax denominator columns carry the 32x V scale
            for h in range(4):
                nc.gpsimd.memset(v_s[:, :, h, 64:65], SCL)
            # DoubleRow zero sub-planes (gpsimd is idle at t0); contiguous
            # per-pt slices
            for pt in range(2):
                nc.gpsimd.memset(kT8[:, pt, 1, :], 0.0)
                nc.gpsimd.memset(qT8[:, pt, 1, :], 0.0)
            # PE warm-up stationary/moving data
            nc.vector.memset(warm_s[:], 0.0)
            # warm the Exp table so LoadActFuncSet is off the critical path
            wexp = small.tile([1, 8], F32, tag="wexp", bufs=1, name="wexp")
            nc.vector.memset(wexp[:], 0.0)
            nc.scalar.activation(out=wexp[:], in_=wexp[:], func=Exp)

            # ---------------- input DMA stream (SP queue order) ----------------
            xq_t = {}  # qb -> [128, 2, 4, 2, 512] tile (hi+lo pair)
            xk_t = {}

            def flat(ap):
                return ap.rearrange("p a b c d -> p (a b c d)")

            def dma_qk(tag, src, store, b):
                t = xin.tile([128, 2, 4, 2, 512], FP8, tag="x", name=f"{tag}{b}")
                for hl in range(2):
                    nc.sync.dma_start(
                        t[:, hl].rearrange("p a b c -> p (a b c)"),
                        src[:, b, 4096 * hl : 4096 * (hl + 1)],
                    )
                store[b] = t

            # wave order is (1, 2, 3, 0): K block 0 + Q block 1 lead
            nc.sync.dma_start(cF_s[:], cF.ap())
            nc.sync.dma_start(flat(cWK_s[:]), cWK.ap())
            dma_qk("xk", xk8, xk_t, 0)
            nc.sync.dma_start(flat(cWQ_s[:]), cWQ.ap())
            dma_qk("xq", xq8, xq_t, 1)
            dma_qk("xk", xk8, xk_t, 1)
            nc.sync.dma_start(flat(cWV_s[:]), cWV.ap())
            nc.sync.dma_start(flat(xv8_s[:, 0]), xv8[:, 0])
            dma_qk("xq", xq8, xq_t, 2)
            dma_qk("xk", xk8, xk_t, 2)
            nc.sync.dma_start(flat(xv8_s[:, 1]), xv8[:, 1])
            dma_qk("xq", xq8, xq_t, 3)
            nc.sync.dma_start(flat(xv8_s[:, 2]), xv8[:, 2])
            dma_qk("xq", xq8, xq_t, 0)
            nc.sync.dma_start(flat(xv8_s[:, 3]), xv8[:, 3])
            dma_qk("xk", xk8, xk_t, 3)
            nc.sync.dma_start(cWO_s[:], cWO.ap())
            nc.sync.dma_start(
                mask4_s[:].rearrange("p h x -> p (h x)"), cMASK.ap()
            )

            # ---------------- PSUM pools (4+2+1+1 banks) --------------------
            # psV/psP sit on top of the stack allocator so they can be
            # released mid-stream into a second 2-bank AV pool (psC2)
            psS = tc.alloc_tile_pool(name="psS", bufs=2, space="PSUM")  # 4 banks
            psC = tc.alloc_tile_pool(name="psC", bufs=2, space="PSUM")  # 2 banks
            psV = tc.alloc_tile_pool(name="psV", bufs=1, space="PSUM")  # 1 bank
            psP = tc.alloc_tile_pool(name="psP", bufs=1, space="PSUM")  # 1 bank

            # PE pstate warm-up: fine-grained dummy matmuls bridge the PE from
            # t~1us until the first real weights land (~5.5us) so the pstate
            # ramp reaches full speed with no idle-gap reset
            _LBL[0] = "warm"
            wps = psP.tile([128, 512], F32, tag="pp", name="wm")
            for i in range(20):
                nc.tensor.matmul(
                    wps[:, 0:256], warm_s[:, 0:128], warm_s[:, 0:256],
                    start=True, stop=True, skip_group_check=True,
                )

            # ---------------- unit emitters ----------------
            def proj_block(w_s, x_t, b_s, outT, pt, b, prologue=False):
                """3-term fp8 DoubleRow projection of one [128, 512] tile:
                x8@W8 + xr8@W8 + x8@W8r, then bias-add writing fp8 qT8/kT8.
                Block 0 runs through the (still idle) scores pool so the four
                b0 tiles pipeline instead of serializing on psP's one bank."""
                _LBL[0] = f"proj_{b}"
                if prologue:
                    ps = psS.tile([128, 2, 512], F32, tag="sc", name=f"pj{pt}{b}")[
                        :, 0, :
                    ]
                else:
                    ps = psP.tile([128, 512], F32, tag="pp", name=f"pj{pt}{b}")
                n = 0
                for xh, wh in [(0, 0), (0, 1), (1, 0)]:  # (x hl, w hl)
                    for kcp in range(4):
                        nc.tensor.matmul(
                            ps[:],
                            w_s[:, wh, kcp, :, 128 * pt : 128 * (pt + 1)],
                            x_t[b][:, xh, kcp],
                            start=(n == 0),
                            stop=(n == 11),
                            perf_mode=DR,
                        )
                        n += 1
                nc.vector.tensor_scalar_add(
                    outT[:, pt, 0, 512 * b : 512 * (b + 1)],
                    ps[:] if not prologue else ps,
                    b_s[:, pt : pt + 1],
                )

            def v_unit(st):
                """V projection of one 128-kpos strip, 3-term fp8 DoubleRow,
                output in natural [kpos, feat] layout scaled by 32."""
                vb, i = st // 4, st % 4
                _LBL[0] = f"v_{st}"
                pv = psV.tile([128, 256], F32, tag="pv", name=f"pv{st}")
                n = 0
                for kcp in range(4):
                    for xh, wh in [(0, 0), (0, 1), (1, 0)]:
                        nc.tensor.matmul(
                            pv[:],
                            xv8_s[:, vb, xh, kcp, :, 128 * i : 128 * (i + 1)],
                            cWV_s[:, wh, kcp],
                            start=(n == 0),
                            stop=(n == 11),
                            perf_mode=DR,
                        )
                        n += 1
                nc.vector.tensor_add(
                    v_s[:, st, :, 0:64],
                    pv[:].rearrange("p (h x) -> p h x", x=64),
                    bvt_s,
                )

            pr_t = {}  # (w, ki) -> probs tile [128, 4, 512] bf16
            ctx_t = {}  # (w, h) -> ctx psum strip [65, 512] f32

            def sc_unit(w, ki):
                """fp8 DoubleRow scores + exp (+ diag mask) for (wave, ktile)."""
                qoff = 128 * max(ki - 4 * w, 0)
                wdt = 512 - qoff
                _LBL[0] = f"sc_{w}_{ki}"
                pr = prp.tile(
                    [128, 4, 512], BF16, tag="pr", bufs=19, name=f"pr_{w}_{ki}"
                )
                pr_t[(w, ki)] = pr
                for hp in range(2):
                    sc = psS.tile([128, 2, 512], F32, tag="sc", name="sc")
                    for j in range(2):
                        h = 2 * hp + j
                        r, pt = 64 * (h % 2), h // 2
                        nc.tensor.matmul(
                            sc[:, j, 0:wdt],
                            kT8[r : r + 64, pt, :, 128 * ki : 128 * (ki + 1)],
                            qT8[r : r + 64, pt, :, 512 * w + qoff : 512 * (w + 1)],
                            start=True,
                            stop=True,
                            perf_mode=DR,
                        )
                    nc.scalar.activation(
                        out=pr[:, 2 * hp : 2 * hp + 2, qoff:512],
                        in_=sc[:, :, 0:wdt],
                        func=Exp,
                        scale=2.0 ** -13,  # 1/8 softmax scale / 32^2 weight scale
                    )
                if ki >= 4 * w:  # diag ktile: mask upper triangle in place
                    MASK_ENG.tensor_mul(
                        pr[:, :, qoff : qoff + 128],
                        pr[:, :, qoff : qoff + 128],
                        mask4_s,
                    )

            wave_pool = {}  # wave -> PSUM pool for its AV/oproj tiles

            def av_open(w, pair):
                for h in (2 * pair, 2 * pair + 1):
                    ctx_t[(w, h)] = wave_pool[w].tile(
                        [65, 512], F32, tag="ctx", name=f"ctx{w}{h}"
                    )

            def av_mm(w, pair, ki):
                """ctx^T[dk+1, q] += V_aug_h^T @ probs, one head pair (bf16).
                Pair-serial (2 PSUM banks) so the scores pool can triple-buffer."""
                qoff = 128 * max(ki - 4 * w, 0)
                last = 4 * w + 3
                _LBL[0] = f"av{pair}_{w}_{ki}"
                for h in (2 * pair, 2 * pair + 1):
                    nc.tensor.matmul(
                        ctx_t[(w, h)][:, qoff:512],
                        v_s[:, ki, h, :],
                        pr_t[(w, ki)][:, h, qoff:512],
                        start=(ki == 0),
                        stop=(ki == last),
                        skip_group_check=True,
                    )

            rbc_t = {}

            def av_fin_recip(w, h):
                """reciprocal + partition-broadcast of head h's denominators."""
                ctx = ctx_t[(w, h)]
                recip = small.tile([1, 512], F32, tag="recip", bufs=4, name="recip")
                nc.vector.reciprocal(recip[:], ctx[64:65, :])
                rbc = small.tile([64, 512], F32, tag="rbc", bufs=4, name="rbc")
                nc.gpsimd.partition_broadcast(rbc[:], recip[:])
                rbc_t[(w, h)] = rbc

            def av_fin_mult(w, h):
                """normalize ctx^T by the broadcast reciprocals."""
                r, pt = 64 * (h % 2), h // 2
                nc.vector.tensor_mul(
                    ctxT_s[r : r + 64, pt, 512 * w : 512 * (w + 1)],
                    ctx_t[(w, h)][0:64, :],
                    rbc_t[(w, h)][:],
                )

            def po_unit(w, u):
                qt, nb = u // 2, u % 2
                st = 4 * w + qt
                _LBL[0] = f"po_{w}_{u}"
                po = wave_pool[w].tile([128, 512], F32, tag="ctx", name="po")
                for fc in range(2):
                    nc.tensor.matmul(
                        po[:],
                        ctxT_s[:, fc, 128 * st : 128 * (st + 1)],
                        cWO_s[:, fc, 512 * nb : 512 * (nb + 1)],
                        start=(fc == 0),
                        stop=(fc == 1),
                    )
                ot = oout.tile([128, 512], BF16, tag="ot", bufs=8, name="ot")
                if w == 0 and u % 2 == 0:
                    # wave 0 is processed last: Act is done with exps by now
                    nc.scalar.activation(
                        out=ot[:], in_=po[:],
                        func=mybir.ActivationFunctionType.Copy,
                    )
                else:
                    nc.vector.tensor_copy(ot[:], po[:])
                nc.sync.dma_start(
                    rs_in[w][128 * qt : 128 * (qt + 1), 512 * nb : 512 * (nb + 1)],
                    ot[:],
                )

            def rs_unit(w):
                import concourse.mybir as mybir_mod

                nc.gpsimd.collective_compute(
                    "ReduceScatter",
                    mybir_mod.AluOpType.add,
                    replica_groups=[[0, 1, 2, 3], [4, 5, 6, 7]],
                    ins=[rs_in[w].opt()],
                    outs=[rs_out[w].opt()],
                )

            # ---------------- budget-paced emission schedule ----------------
            # sc units are emitted eagerly (psS double-buffering paces them to
            # the Act engine's exp rate at runtime); filler PE work pops from
            # a FIFO, gated on DMA arrival (expressed as a min sc-index) and
            # budgeted so at most ~one exp-time of filler sits between
            # consecutive sc units in the PE queue.
            def projK(pt, b, prologue=False):
                proj_block(cWK_s, xk_t, bk_s, kT8, pt, b, prologue)

            def projQ(pt, b, prologue=False):
                proj_block(cWQ_s, xq_t, bq_s, qT8, pt, b, prologue)

            # wave order: shortest wave (0) LAST so the post-exp tail (pair-1
            # AV catch-up + fins + oproj + final RS) is as small as possible
            WAVES = (1, 2, 3, 0)
            sc_list = [(w, ki) for w in WAVES for ki in range(4 * w + 4)]
            sc_idx = {u: i for i, u in enumerate(sc_list)}
            NSC = len(sc_list)

            def wdt_of(w, ki):
                return 512 - 128 * max(ki - 4 * w, 0)

            def act_cost(w, ki):  # us of Act work per sc unit
                return (4 * wdt_of(w, ki) * 0.833 + 2 * 185) / 1000.0

            def sc_pe_cost(w, ki):  # us of PE work per sc unit (fp8 DR)
                return 2 * wdt_of(w, ki) * 0.4167 / 1000.0

            fillers = []  # (pe_cost_us, gate_sc_index, deadline_sc_index, fn)

            def F(cost, gate, deadline, fn, *a):
                fillers.append((cost, gate, deadline, lambda a=a: fn(*a)))

            PJ, VU, AV = 1.28, 0.64, 0.4167 / 1000.0

            def wave_entries(w, rs_prev):
                """AV (pair-serial), fins, oproj and prior-wave RS for wave w."""
                F(0.0, None, None, av_open, w, 0)
                for ki in range(4 * w + 4):
                    F(
                        2 * wdt_of(w, ki) * AV,
                        min(sc_idx[(w, ki)] + 3, NSC),
                        None,
                        av_mm, w, 0, ki,
                    )
                    if ki == 1 and rs_prev is not None:
                        # prior wave's RS: emitted a couple of units into the
                        # next wave so its semwait never head-blocks the Pool
                        # queue ahead of this wave's broadcasts
                        F(0.0, None, None, rs_unit, rs_prev)
                for h in (0, 1):
                    F(0.0, None, None, av_fin_recip, w, h)
                for h in (0, 1):
                    F(0.0, None, None, av_fin_mult, w, h)
                F(0.0, None, None, av_open, w, 1)
                for ki in range(4 * w + 4):
                    F(
                        2 * wdt_of(w, ki) * AV,
                        min(sc_idx[(w, ki)] + 3, NSC),
                        None,
                        av_mm, w, 1, ki,
                    )
                for h in (2, 3):
                    F(0.0, None, None, av_fin_recip, w, h)
                for h in (2, 3):
                    F(0.0, None, None, av_fin_mult, w, h)
                g0 = min(sc_idx[(w, 4 * w + 3)] + 4, NSC)
                for u in range(8):
                    F(0.43,
                      min(g0 + u // 2, NSC) if w != WAVES[-1] else None,
                      None, po_unit, w, u)

            def open_psC2():
                psP.release()
                psV.release()
                wave_pool[3] = tc.alloc_tile_pool(
                    name="psC2", bufs=2, space="PSUM"
                )  # 2 banks: wave 3's AV/oproj overlap wave 2's drain

            wave_pool[1] = psC
            wave_pool[2] = psC
            wave_pool[0] = psC

            # FIFO interleaves wave-1's AV/po entries EARLY so its RS fires
            # ~40us in; later proj/V fillers sit between wave sections.
            # gates track DMA arrival; deadlines = first reader's sc index.
            for pt in range(2):
                F(PJ, 1, sc_idx[(1, 4)], projK, pt, 1)
            for st in (0, 1, 2, 3):
                F(VU, 3, sc_idx[(1, 0)] + 3, v_unit, st)
            for pt in range(2):
                F(PJ, 4, sc_idx[(2, 0)], projQ, pt, 2)
            for pt in range(2):
                F(PJ, 5, sc_idx[(2, 8)], projK, pt, 2)
            for st in (4, 5, 6, 7):
                F(VU, 7, sc_idx[(1, 4)] + 3, v_unit, st)
            wave_entries(1, None)
            for pt in range(2):
                F(PJ, 8, sc_idx[(3, 0)], projQ, pt, 3)
            for st in (8, 9, 10, 11):
                F(VU, 10, sc_idx[(2, 8)] + 3, v_unit, st)
            for pt in range(2):
                F(PJ, 12, sc_idx[(0, 0)], projQ, pt, 0)
            for pt in range(2):
                F(PJ, 14, sc_idx[(3, 12)], projK, pt, 3)
            for st in (12, 13, 14, 15):
                F(VU, 13, sc_idx[(3, 12)] + 3, v_unit, st)
            wave_entries(2, 1)
            F(0.0, None, sc_idx[(3, 0)] + 3, open_psC2)
            wave_entries(3, 2)
            wave_entries(0, 3)
            F(0.0, None, None, rs_unit, WAVES[-1])

            state = {"budget": 0.0, "sci": 0}

            def pump(force=False):
                while fillers:
                    cost, gate, deadline, fn = fillers[0]
                    due = deadline is not None and state["sci"] >= deadline
                    if not due:
                        if gate is not None and state["sci"] < gate:
                            break
                        if not force and state["budget"] < cost:
                            break
                    fillers.pop(0)
                    fn()
                    state["budget"] -= cost

            # prologue: K block 0 + Q block 1 projections ahead of sc(1, 0)
            projK(0, 0, prologue=True); projK(1, 0, prologue=True)
            projQ(0, 1, prologue=True); projQ(1, 1, prologue=True)
            for w, ki in sc_list:
                pump()  # fire deadline-due prerequisites before this sc unit
                sc_unit(w, ki)
                state["sci"] += 1
                state["budget"] += 1.25 * act_cost(w, ki) - sc_pe_cost(w, ki)
                pump()
            pump(force=True)

            # final output DMAs (after all RS so the SP queue never head-blocks)
            for w in range(4):
                nc.sync.dma_start(out[128 * w : 128 * (w + 1), :], rs_out[w][:])
            wave_pool[3].release()  # psC2
            psC.release()
            psS.release()

    nc.compile()
    return nc


def _prep_inputs(query, key_, value, w_q, b_q, w_k, b_k, w_v, b_v, w_o, b_o):
    """Build the 8 per-core input maps (host-side sharding / fp8 packing)."""
    import ml_dtypes

    f32 = np.float32
    bf16 = ml_dtypes.bfloat16
    e4 = ml_dtypes.float8_e4m3

    def fp8_pair(a):
        hi = a.astype(e4)
        lo = (a - hi.astype(f32)).astype(e4)
        return hi, lo

    def pack_w(wT_scaled):
        # [1024, 256] -> [p, (hl, kcp, t, feat)] = [128, 4096]
        hi, lo = fp8_pair(wT_scaled)
        out = np.empty((128, 2, 4, 2, 256), e4)
        for i, a in enumerate((hi, lo)):
            out[:, i] = a.reshape(4, 2, 128, 256).transpose(2, 0, 1, 3)
        return np.ascontiguousarray(out).reshape(128, 4096)

    def pack_x(xT):
        # [1024, 2048] -> [p, blk, (hl, kcp, t, c)] = [128, 4, 8192]
        hi, lo = fp8_pair(xT)
        out = np.empty((128, 4, 2, 4, 2, 512), e4)
        for i, a in enumerate((hi, lo)):
            # [kcp, t, p, blk, c] -> [p, blk, kcp, t, c]
            out[:, :, i] = a.reshape(4, 2, 128, 4, 512).transpose(2, 3, 0, 1, 4)
        return np.ascontiguousarray(out).reshape(128, 4, 8192)

    r = np.arange(128)
    mask = (r[None, :] >= r[:, None]).astype(f32)  # [kpos, q] allowed: q >= k
    mask4 = np.ascontiguousarray(np.tile(mask, (1, 4)).astype(bf16))

    wqT = np.asarray(w_q, f32).T * SCL
    wkT = np.asarray(w_k, f32).T * SCL
    wvT = np.asarray(w_v, f32).T * SCL
    woT = np.asarray(w_o, f32).T

    xP = {}
    for g in range(B):
        for nm, src in (("q", query), ("k", key_), ("v", value)):
            xT = np.ascontiguousarray(np.asarray(src[g], f32).T)
            xP[(nm, g)] = pack_x(xT)

    in_maps = []
    for c in range(N_CORES):
        g, p = c // 4, c % 4
        fsel = slice(FPC * p, FPC * (p + 1))
        woc = (
            np.ascontiguousarray(
                woT[fsel, :].reshape(2, 128, D).transpose(1, 0, 2)
            ).astype(bf16)
        )
        bq_c = (np.asarray(b_q, f32)[fsel] * SCL).reshape(2, 128).T
        bk_c = (np.asarray(b_k, f32)[fsel] * SCL).reshape(2, 128).T
        bvt = np.broadcast_to(np.asarray(b_v, f32)[fsel] * SCL, (128, FPC))
        cF_arr = np.concatenate([bq_c, bk_c, bvt], axis=1)
        in_maps.append(
            {
                "cWQ": pack_w(wqT[:, fsel]),
                "cWK": pack_w(wkT[:, fsel]),
                "cWV": pack_w(wvT[:, fsel]),
                "cWO": woc,
                "cMASK": mask4,
                "cF": np.ascontiguousarray(cF_arr.astype(f32)),
                "xq8": xP[("q", g)],
                "xk8": xP[("k", g)],
                "xv8": xP[("v", g)],
            }
        )
    return in_maps


def run(inputs, trace=False):
    from concourse.bass_utils import run_bass_kernel_spmd

    if "nc" not in _CACHE:
        _CACHE["nc"] = _build_nc()
    nc = _CACHE["nc"]
    in_maps = _prep_inputs(
        inputs["query"], inputs["key_"], inputs["value"],
        inputs["w_q"], inputs["b_q"], inputs["w_k"], inputs["b_k"],
        inputs["w_v"], inputs["b_v"], inputs["w_o"], inputs["b_o"],
    )
    res = run_bass_kernel_spmd(
        nc, in_maps, core_ids=list(range(N_CORES)), trace=trace,
    )
    bo = np.asarray(inputs["b_o"], np.float32)
    out = np.empty((B, S, D), np.float32)
    for c in range(N_CORES):
        g, p = c // 4, c % 4
        # RS for wave w scatters q rows [512w + 128p, 512w + 128(p+1))
        core_out = np.asarray(res.results[c]["out"], np.float32)
        for w in range(4):
            out[g, 512 * w + 128 * p : 512 * w + 128 * (p + 1), :] = (
                core_out[128 * w : 128 * (w + 1)] + bo
            )
    return out, res


def kernel(**inputs):
    out, _ = run(inputs, trace=False)
    return out


# revision 48
# speedup vs baseline: 1.0052x; 1.0052x over previous
"""Causal multi-head attention on 8 Trainium2 NeuronCores — v3 (fp8 DoubleRow).

Sharding: core c -> (batch g = c // 4, head-group p = c % 4, heads 4p..4p+3).

v3 moves the Q/K/V projections and the QK^T scores to fp8e4m3 DoubleRow
matmuls (0.5 PE cycles per moving column vs 1.0 for bf16):

- Inputs arrive from host as fp8 hi/lo pairs (x8 = fp8(x), xr8 = fp8(x - x8))
  laid out [128, hl, blk, kcp, t, 512] so each DoubleRow matmul contracts
  2x128 input features per instruction. Projections compute the 3-term
  correction x8@W8 + xr8@W8 + x8@W8r (error ~0.1%), 12 DR matmuls per
  [128, 512] output tile vs 16 bf16 matmuls.
- Weights are host-scaled by 32 (w' = 32w) so fp8 quantization stays clear
  of the subnormal floor; the 32^2 factor on scores is folded into the exp
  scale (2^-13), and the 32 on V is cancelled by a 32.0 ones-column that
  also produces the softmax denominators.
- Q/K land directly in fp8 [128, pt, 2, S] with a zeroed DoubleRow sub-plane
  (the dk=64 contraction is packed as 64 real + 64 zero rows), so scores run
  DoubleRow at 0.5 cyc/col with no extra layout work. AV + O-proj stay bf16
  (probs/ctx quantization would blow the 2e-2 error budget).

Scheduling: projections stream through 2 PSUM banks and overlap the early
score/exp waves (Act starts ~10us in); AV opens after proj+V release their
banks (~27us) and catches up from the 19-deep probs buffer. Output is
ReduceScattered per 512-row wave (4 collectives) so only the last wave's
RS is exposed at the tail. PE warm-up dummies hold the pstate ramp.
"""

import numpy as np

B, S, D, H = 2, 2048, 1024, 16
DK = D // H
N_CORES = 8
FPC = 256  # features (head dims) per core
SCL = 32.0  # host-side weight scale

_CACHE = {}
MM_LABELS = []  # diagnostic: label of each emitted matmul, in emission order
_LBL = ["?"]


def _build_nc():
    import concourse.mybir as mybir
    import concourse.tile as tile
    from concourse import bacc

    F32 = mybir.dt.float32
    BF16 = mybir.dt.bfloat16
    FP8 = mybir.dt.float8e4
    Exp = mybir.ActivationFunctionType.Exp
    DR = mybir.MatmulPerfMode.DoubleRow

    nc = bacc.Bacc("TRN2", target_bir_lowering=False, debug=False, num_devices=8)
    import os as _os
    MASK_ENG = nc.gpsimd if not _os.environ.get("K_MASK_DVE") else nc.vector

    MM_LABELS.clear()
    _orig_mm = nc.tensor.matmul

    def _mm(*a, **kw):
        MM_LABELS.append(_LBL[0])
        return _orig_mm(*a, **kw)

    nc.tensor.matmul = _mm

    # weights: flat rows of [hl, kcp, t, feat] fp8 (hi/lo), value = fp8(32*w.T)
    # (flat innermost dim keeps DMA descriptors at 4KB, off the
    # min-transfer-time floor)
    cWQ = nc.dram_tensor("cWQ", [128, 4096], FP8, kind="ExternalInput")
    cWK = nc.dram_tensor("cWK", [128, 4096], FP8, kind="ExternalInput")
    cWV = nc.dram_tensor("cWV", [128, 4096], FP8, kind="ExternalInput")
    cWO = nc.dram_tensor("cWO", [128, 2, 1024], BF16, kind="ExternalInput")
    cMASK = nc.dram_tensor("cMASK", [128, 512], BF16, kind="ExternalInput")
    cF = nc.dram_tensor("cF", [128, 260], F32, kind="ExternalInput")
    # activations: flat per-block rows of [hl, kcp, t, c] fp8
    # (x[256*kcp+128*t+p, 512*blk+c]); one 8KB/partition DMA per block
    xq8 = nc.dram_tensor("xq8", [128, 4, 8192], FP8, kind="ExternalInput")
    xk8 = nc.dram_tensor("xk8", [128, 4, 8192], FP8, kind="ExternalInput")
    xv8 = nc.dram_tensor("xv8", [128, 4, 8192], FP8, kind="ExternalInput")
    out = nc.dram_tensor("out", [512, D], BF16, kind="ExternalOutput")

    with tile.TileContext(nc) as tc:
        with (
            tc.tile_pool(name="consts", bufs=1) as consts,
            tc.tile_pool(name="persist", bufs=1) as persist,
            tc.tile_pool(name="xin", bufs=3) as xin,
            tc.tile_pool(name="prp", bufs=1) as prp,
            tc.tile_pool(name="small", bufs=3) as small,
            tc.tile_pool(name="oout", bufs=6) as oout,
            tc.tile_pool(name="dram", bufs=1, space="DRAM") as dram,
        ):
            # ---------------- constants ----------------
            cWQ_s = consts.tile([128, 2, 4, 2, 256], FP8, tag="cWQ", name="cWQ_s")
            cWK_s = consts.tile([128, 2, 4, 2, 256], FP8, tag="cWK", name="cWK_s")
            cWV_s = consts.tile([128, 2, 4, 2, 256], FP8, tag="cWV", name="cWV_s")
            cWO_s = consts.tile([128, 2, 1024], BF16, tag="cWO", name="cWO_s")
            mask4_s = consts.tile([128, 4, 128], BF16, tag="mask", name="mask4_s")
            cF_s = consts.tile([128, 260], F32, tag="cF", name="cF_s")
            warm_s = consts.tile([128, 512], BF16, tag="warm", name="warm_s")
            bq_s = cF_s[:, 0:2]
            bk_s = cF_s[:, 2:4]
            bvt_s = cF_s[:, 4:260].rearrange("p (h x) -> p h x", h=4)

            # ---------------- persistent activations ----------------
            # q/k in fp8, [p, pt, j, col]: j=0 data, j=1 zeros (DoubleRow pad)
            qT8 = persist.tile([128, 2, 2, S], FP8, tag="qT8", name="qT8")
            kT8 = persist.tile([128, 2, 2, S], FP8, tag="kT8", name="kT8")
            xv8_s = persist.tile([128, 4, 2, 4, 2, 512], FP8, tag="xv8", name="xv8_s")
            v_s = persist.tile([128, 16, 4, 65], BF16, tag="v", name="v_s")
            ctxT_s = persist.tile([128, 2, S], BF16, tag="ctxT", name="ctxT_s")

            rs_in = [dram.tile([512, D], BF16, name=f"rs_in{w}") for w in range(4)]
            rs_out = [dram.tile([128, D], BF16, name=f"rs_out{w}") for w in range(4)]

            # ---------------- t=0 setup ----------------
            # softmax denominator columns carry the 32x V scale
            for h in range(4):
                nc.gpsimd.memset(v_s[:, :, h, 64:65], SCL)
            # DoubleRow zero sub-planes (gpsimd is idle at t0); contiguous
            # per-pt slices
            for pt in range(2):
                nc.gpsimd.memset(kT8[:, pt, 1, :], 0.0)
                nc.gpsimd.memset(qT8[:, pt, 1, :], 0.0)
            # PE warm-up stationary/moving data
            nc.vector.memset(warm_s[:], 0.0)
            # warm the Exp table so LoadActFuncSet is off the critical path
            wexp = small.tile([1, 8], F32, tag="wexp", bufs=1, name="wexp")
            nc.vector.memset(wexp[:], 0.0)
            nc.scalar.activation(out=wexp[:], in_=wexp[:], func=Exp)

            # ---------------- input DMA stream (SP queue order) ----------------
            xq_t = {}  # qb -> [128, 2, 4, 2, 512] tile (hi+lo pair)
            xk_t = {}

            def flat(ap):
                return ap.rearrange("p a b c d -> p (a b c d)")

            def dma_qk(tag, src, store, b):
                t = xin.tile([128, 2, 4, 2, 512], FP8, tag="x", name=f"{tag}{b}")
                for hl in range(2):
                    nc.sync.dma_start(
                        t[:, hl].rearrange("p a b c -> p (a b c)"),
                        src[:, b, 4096 * hl : 4096 * (hl + 1)],
                    )
                store[b] = t

            # wave order is (1, 2, 3, 0): K block 0 + Q block 1 lead
            nc.sync.dma_start(cF_s[:], cF.ap())
            nc.sync.dma_start(flat(cWK_s[:]), cWK.ap())
            dma_qk("xk", xk8, xk_t, 0)
            nc.sync.dma_start(flat(cWQ_s[:]), cWQ.ap())
            dma_qk("xq", xq8, xq_t, 1)
            dma_qk("xk", xk8, xk_t, 1)
            nc.sync.dma_start(flat(cWV_s[:]), cWV.ap())
            nc.sync.dma_start(flat(xv8_s[:, 0]), xv8[:, 0])
            dma_qk("xq", xq8, xq_t, 2)
            dma_qk("xk", xk8, xk_t, 2)
            nc.sync.dma_start(flat(xv8_s[:, 1]), xv8[:, 1])
            dma_qk("xq", xq8, xq_t, 3)
            nc.sync.dma_start(flat(xv8_s[:, 2]), xv8[:, 2])
            dma_qk("xq", xq8, xq_t, 0)
            nc.sync.dma_start(flat(xv8_s[:, 3]), xv8[:, 3])
            dma_qk("xk", xk8, xk_t, 3)
            nc.sync.dma_start(cWO_s[:], cWO.ap())
            nc.sync.dma_start(
                mask4_s[:].rearrange("p h x -> p (h x)"), cMASK.ap()
            )

            # ---------------- PSUM pools (4+2+1+1 banks) --------------------
            # psV/psP sit on top of the stack allocator so they can be
            # released mid-stream into a second 2-bank AV pool (psC2)
            psS = tc.alloc_tile_pool(name="psS", bufs=2, space="PSUM")  # 4 banks
            psC = tc.alloc_tile_pool(name="psC", bufs=2, space="PSUM")  # 2 banks
            psV = tc.alloc_tile_pool(name="psV", bufs=1, space="PSUM")  # 1 bank
            psP = tc.alloc_tile_pool(name="psP", bufs=1, space="PSUM")  # 1 bank

            # PE pstate warm-up: fine-grained dummy matmuls bridge the PE from
            # t~1us until the first real weights land (~5.5us) so the pstate
            # ramp reaches full speed with no idle-gap reset
            _LBL[0] = "warm"
            wps = psP.tile([128, 512], F32, tag="pp", name="wm")
            for i in range(20):
                nc.tensor.matmul(
                    wps[:, 0:256], warm_s[:, 0:128], warm_s[:, 0:256],
                    start=True, stop=True, skip_group_check=True,
                )

            # ---------------- unit emitters ----------------
            def proj_block(w_s, x_t, b_s, outT, pt, b, prologue=False):
                """3-term fp8 DoubleRow projection of one [128, 512] tile:
                x8@W8 + xr8@W8 + x8@W8r, then bias-add writing fp8 qT8/kT8.
                Block 0 runs through the (still idle) scores pool so the four
                b0 tiles pipeline instead of serializing on psP's one bank."""
                _LBL[0] = f"proj_{b}"
                if prologue:
                    ps = psS.tile([128, 2, 512], F32, tag="sc", name=f"pj{pt}{b}")[
                        :, 0, :
                    ]
                else:
                    ps = psP.tile([128, 512], F32, tag="pp", name=f"pj{pt}{b}")
                n = 0
                for xh, wh in [(0, 0), (0, 1), (1, 0)]:  # (x hl, w hl)
                    for kcp in range(4):
                        nc.tensor.matmul(
                            ps[:],
                            w_s[:, wh, kcp, :, 128 * pt : 128 * (pt + 1)],
                            x_t[b][:, xh, kcp],
                            start=(n == 0),
                            stop=(n == 11),
                            perf_mode=DR,
                        )
                        n += 1
                nc.vector.tensor_scalar_add(
                    outT[:, pt, 0, 512 * b : 512 * (b + 1)],
                    ps[:] if not prologue else ps,
                    b_s[:, pt : pt + 1],
                )

            def v_unit(st):
                """V projection of one 128-kpos strip, 3-term fp8 DoubleRow,
                output in natural [kpos, feat] layout scaled by 32."""
                vb, i = st // 4, st % 4
                _LBL[0] = f"v_{st}"
                pv = psV.tile([128, 256], F32, tag="pv", name=f"pv{st}")
                n = 0
                for kcp in range(4):
                    for xh, wh in [(0, 0), (0, 1), (1, 0)]:
                        nc.tensor.matmul(
                            pv[:],
                            xv8_s[:, vb, xh, kcp, :, 128 * i : 128 * (i + 1)],
                            cWV_s[:, wh, kcp],
                            start=(n == 0),
                            stop=(n == 11),
                            perf_mode=DR,
                        )
                        n += 1
                nc.vector.tensor_add(
                    v_s[:, st, :, 0:64],
                    pv[:].rearrange("p (h x) -> p h x", x=64),
                    bvt_s,
                )

            pr_t = {}  # (w, ki) -> probs tile [128, 4, 512] bf16
            ctx_t = {}  # (w, h) -> ctx psum strip [65, 512] f32

            def sc_unit(w, ki):
                """fp8 DoubleRow scores + exp (+ diag mask) for (wave, ktile)."""
                qoff = 128 * max(ki - 4 * w, 0)
                wdt = 512 - qoff
                _LBL[0] = f"sc_{w}_{ki}"
                pr = prp.tile(
                    [128, 4, 512], BF16, tag="pr", bufs=19, name=f"pr_{w}_{ki}"
                )
                pr_t[(w, ki)] = pr
                for hp in range(2):
                    sc = psS.tile([128, 2, 512], F32, tag="sc", name="sc")
                    for j in range(2):
                        h = 2 * hp + j
                        r, pt = 64 * (h % 2), h // 2
                        nc.tensor.matmul(
                            sc[:, j, 0:wdt],
                            kT8[r : r + 64, pt, :, 128 * ki : 128 * (ki + 1)],
                            qT8[r : r + 64, pt, :, 512 * w + qoff : 512 * (w + 1)],
                            start=True,
                            stop=True,
                            perf_mode=DR,
                        )
                    nc.scalar.activation(
                        out=pr[:, 2 * hp : 2 * hp + 2, qoff:512],
                        in_=sc[:, :, 0:wdt],
                        func=Exp,
                        scale=2.0 ** -13,  # 1/8 softmax scale / 32^2 weight scale
                    )
                if ki >= 4 * w:  # diag ktile: mask upper triangle in place
                    MASK_ENG.tensor_mul(
                        pr[:, :, qoff : qoff + 128],
                        pr[:, :, qoff : qoff + 128],
                        mask4_s,
                    )

            wave_pool = {}  # wave -> PSUM pool for its AV/oproj tiles

            def av_open(w, pair):
                for h in (2 * pair, 2 * pair + 1):
                    ctx_t[(w, h)] = wave_pool[w].tile(
                        [65, 512], F32, tag="ctx", name=f"ctx{w}{h}"
                    )

            def av_mm(w, pair, ki):
                """ctx^T[dk+1, q] += V_aug_h^T @ probs, one head pair (bf16).
                Pair-serial (2 PSUM banks) so the scores pool can triple-buffer."""
                qoff = 128 * max(ki - 4 * w, 0)
                last = 4 * w + 3
                _LBL[0] = f"av{pair}_{w}_{ki}"
                for h in (2 * pair, 2 * pair + 1):
                    nc.tensor.matmul(
                        ctx_t[(w, h)][:, qoff:512],
                        v_s[:, ki, h, :],
                        pr_t[(w, ki)][:, h, qoff:512],
                        start=(ki == 0),
                        stop=(ki == last),
                        skip_group_check=True,
                    )

            rbc_t = {}

            def av_fin_recip(w, h):
                """reciprocal + partition-broadcast of head h's denominators."""
                ctx = ctx_t[(w, h)]
                recip = small.tile([1, 512], F32, tag="recip", bufs=4, name="recip")
                nc.vector.reciprocal(recip[:], ctx[64:65, :])
                rbc = small.tile([64, 512], F32, tag="rbc", bufs=4, name="rbc")
                nc.gpsimd.partition_broadcast(rbc[:], recip[:])
                rbc_t[(w, h)] = rbc

            def av_fin_mult(w, h):
                """normalize ctx^T by the broadcast reciprocals."""
                r, pt = 64 * (h % 2), h // 2
                nc.vector.tensor_mul(
                    ctxT_s[r : r + 64, pt, 512 * w : 512 * (w + 1)],
                    ctx_t[(w, h)][0:64, :],
                    rbc_t[(w, h)][:],
                )

            def po_unit(w, u):
                qt, nb = u // 2, u % 2
                st = 4 * w + qt
                _LBL[0] = f"po_{w}_{u}"
                po = wave_pool[w].tile([128, 512], F32, tag="ctx", name="po")
                for fc in range(2):
                    nc.tensor.matmul(
                        po[:],
                        ctxT_s[:, fc, 128 * st : 128 * (st + 1)],
                        cWO_s[:, fc, 512 * nb : 512 * (nb + 1)],
                        start=(fc == 0),
                        stop=(fc == 1),
                    )
                ot = oout.tile([128, 512], BF16, tag="ot", bufs=8, name="ot")
                if w == 0 and u % 2 == 0:
                    # wave 0 is processed last: Act is done with exps by now
                    nc.scalar.activation(
                        out=ot[:], in_=po[:],
                        func=mybir.ActivationFunctionType.Copy,
                    )
                else:
                    nc.vector.tensor_copy(ot[:], po[:])
                nc.sync.dma_start(
                    rs_in[w][128 * qt : 128 * (qt + 1), 512 * nb : 512 * (nb + 1)],
                    ot[:],
                )

            def rs_unit(w):
                import concourse.mybir as mybir_mod

                nc.gpsimd.collective_compute(
                    "ReduceScatter",
                    mybir_mod.AluOpType.add,
                    replica_groups=[[0, 1, 2, 3], [4, 5, 6, 7]],
                    ins=[rs_in[w].opt()],
                    outs=[rs_out[w].opt()],
                )

            # ---------------- budget-paced emission schedule ----------------
            # sc units are emitted eagerly (psS double-buffering paces them to
            # the Act engine's exp rate at runtime); filler PE work pops from
            # a FIFO, gated on DMA arrival (expressed as a min sc-index) and
            # budgeted so at most ~one exp-time of filler sits between
            # consecutive sc units in the PE queue.
            def projK(pt, b, prologue=False):
                proj_block(cWK_s, xk_t, bk_s, kT8, pt, b, prologue)

            def projQ(pt, b, prologue=False):
                proj_block(cWQ_s, xq_t, bq_s, qT8, pt, b, prologue)

            # wave order: shortest wave (0) LAST so the post-exp tail (pair-1
            # AV catch-up + fins + oproj + final RS) is as small as possible
            WAVES = (1, 2, 3, 0)
            sc_list = [(w, ki) for w in WAVES for ki in range(4 * w + 4)]
            sc_idx = {u: i for i, u in enumerate(sc_list)}
            NSC = len(sc_list)

            def wdt_of(w, ki):
                return 512 - 128 * max(ki - 4 * w, 0)

            def act_cost(w, ki):  # us of Act work per sc unit
                return (4 * wdt_of(w, ki) * 0.833 + 2 * 185) / 1000.0

            def sc_pe_cost(w, ki):  # us of PE work per sc unit (fp8 DR)
                return 2 * wdt_of(w, ki) * 0.4167 / 1000.0

            fillers = []  # (pe_cost_us, gate_sc_index, deadline_sc_index, fn)

            def F(cost, gate, deadline, fn, *a):
                fillers.append((cost, gate, deadline, lambda a=a: fn(*a)))

            PJ, VU, AV = 1.28, 0.64, 0.4167 / 1000.0

            def wave_entries(w, rs_prev):
                """AV (pair-serial), fins, oproj and prior-wave RS for wave w."""
                F(0.0, None, None, av_open, w, 0)
                for ki in range(4 * w + 4):
                    F(
                        2 * wdt_of(w, ki) * AV,
                        min(sc_idx[(w, ki)] + 3, NSC),
                        None,
                        av_mm, w, 0, ki,
                    )
                    if ki == 1 and rs_prev is not None:
                        # prior wave's RS: emitted a couple of units into the
                        # next wave so its semwait never head-blocks the Pool
                        # queue ahead of this wave's broadcasts
                        F(0.0, None, None, rs_unit, rs_prev)
                for h in (0, 1):
                    F(0.0, None, None, av_fin_recip, w, h)
                for h in (0, 1):
                    F(0.0, None, None, av_fin_mult, w, h)
                F(0.0, None, None, av_open, w, 1)
                for ki in range(4 * w + 4):
                    F(
                        2 * wdt_of(w, ki) * AV,
                        min(sc_idx[(w, ki)] + 3, NSC),
                        None,
                        av_mm, w, 1, ki,
                    )
                for h in (2, 3):
                    F(0.0, None, None, av_fin_recip, w, h)
                for h in (2, 3):
                    F(0.0, None, None, av_fin_mult, w, h)
                g0 = min(sc_idx[(w, 4 * w + 3)] + 4, NSC)
                for u in range(8):
                    F(0.43,
                      min(g0 + u // 2, NSC) if w != WAVES[-1] else None,
                      None, po_unit, w, u)

            def open_psC2():
                psP.release()
                psV.release()
                wave_pool[3] = tc.alloc_tile_pool(
                    name="psC2", bufs=2, space="PSUM"
                )  # 2 banks: wave 3's AV/oproj overlap wave 2's drain

            wave_pool[1] = psC
            wave_pool[2] = psC
            wave_pool[0] = psC

            # FIFO interleaves wave-1's AV/po entries EARLY so its RS fires
            # ~40us in; later proj/V fillers sit between wave sections.
            # gates track DMA arrival; deadlines = first reader's sc index.
            for pt in range(2):
                F(PJ, 1, sc_idx[(1, 4)], projK, pt, 1)
            for st in (0, 1, 2, 3):
                F(VU, 3, sc_idx[(1, 0)] + 3, v_unit, st)
            for pt in range(2):
                F(PJ, 4, sc_idx[(2, 0)], projQ, pt, 2)
            for pt in range(2):
                F(PJ, 5, sc_idx[(2, 8)], projK, pt, 2)
            for st in (4, 5, 6, 7):
                F(VU, 7, sc_idx[(1, 4)] + 3, v_unit, st)
            wave_entries(1, None)
            for pt in range(2):
                F(PJ, 8, sc_idx[(3, 0)], projQ, pt, 3)
            for st in (8, 9, 10, 11):
                F(VU, 10, sc_idx[(2, 8)] + 3, v_unit, st)
            for pt in range(2):
                F(PJ, 12, sc_idx[(0, 0)], projQ, pt, 0)
            for pt in range(2):
                F(PJ, 14, sc_idx[(3, 12)], projK, pt, 3)
            for st in (12, 13, 14, 15):
                F(VU, 13, sc_idx[(3, 12)] + 3, v_unit, st)
            wave_entries(2, 1)
            F(0.0, None, sc_idx[(3, 0)] + 3, open_psC2)
            wave_entries(3, 2)
            wave_entries(0, 3)
            F(0.0, None, None, rs_unit, WAVES[-1])

            state = {"budget": 0.0, "sci": 0}

            def pump(force=False):
                while fillers:
                    cost, gate, deadline, fn = fillers[0]
                    due = deadline is not None and state["sci"] >= deadline
                    if not due:
                        if gate is not None and state["sci"] < gate:
                            break
                        if not force and state["budget"] < cost:
                            break
                    fillers.pop(0)
                    fn()
                    state["budget"] -= cost

            # prologue: K block 0 + Q block 1 projections ahead of sc(1, 0)
            projK(0, 0, prologue=True); projK(1, 0, prologue=True)
            projQ(0, 1, prologue=True); projQ(1, 1, prologue=True)
            for w, ki in sc_list:
                pump()  # fire deadline-due prerequisites before this sc unit
                sc_unit(w, ki)
                state["sci"] += 1
                state["budget"] += 1.25 * act_cost(w, ki) - sc_pe_cost(w, ki)
                pump()
            pump(force=True)

            # final output DMAs (after all RS so the SP queue never head-blocks)
            for w in range(4):
                nc.sync.dma_start(out[128 * w : 128 * (w + 1), :], rs_out[w][:])
            wave_pool[3].release()  # psC2
            psC.release()
            psS.release()

    nc.compile()
    return nc


def _prep_inputs(query, key_, value, w_q, b_q, w_k, b_k, w_v, b_v, w_o, b_o):
    """Build the 8 per-core input maps (host-side sharding / fp8 packing)."""
    import ml_dtypes

    f32 = np.float32
    bf16 = ml_dtypes.bfloat16
    e4 = ml_dtypes.float8_e4m3

    def fp8_pair(a):
        hi = a.astype(e4)
        lo = (a - hi.astype(f32)).astype(e4)
        return hi, lo

    def pack_w(wT_scaled):
        # [1024, 256] -> [p, (hl, kcp, t, feat)] = [128, 4096]
        hi, lo = fp8_pair(wT_scaled)
        out = np.empty((128, 2, 4, 2, 256), e4)
        for i, a in enumerate((hi, lo)):
            out[:, i] = a.reshape(4, 2, 128, 256).transpose(2, 0, 1, 3)
        return np.ascontiguousarray(out).reshape(128, 4096)

    def pack_x(xT):
        # [1024, 2048] -> [p, blk, (hl, kcp, t, c)] = [128, 4, 8192]
        hi, lo = fp8_pair(xT)
        out = np.empty((128, 4, 2, 4, 2, 512), e4)
        for i, a in enumerate((hi, lo)):
            # [kcp, t, p, blk, c] -> [p, blk, kcp, t, c]
            out[:, :, i] = a.reshape(4, 2, 128, 4, 512).transpose(2, 3, 0, 1, 4)
        return np.ascontiguousarray(out).reshape(128, 4, 8192)

    r = np.arange(128)
    mask = (r[None, :] >= r[:, None]).astype(f32)  # [kpos, q] allowed: q >= k
    mask4 = np.ascontiguousarray(np.tile(mask, (1, 4)).astype(bf16))

    wqT = np.asarray(w_q, f32).T * SCL
    wkT = np.asarray(w_k, f32).T * SCL
    wvT = np.asarray(w_v, f32).T * SCL
    woT = np.asarray(w_o, f32).T

    xP = {}
    for g in range(B):
        for nm, src in (("q", query), ("k", key_), ("v", value)):
            xT = np.ascontiguousarray(np.asarray(src[g], f32).T)
            xP[(nm, g)] = pack_x(xT)

    in_maps = []
    for c in range(N_CORES):
        g, p = c // 4, c % 4
        fsel = slice(FPC * p, FPC * (p + 1))
        woc = (
            np.ascontiguousarray(
                woT[fsel, :].reshape(2, 128, D).transpose(1, 0, 2)
            ).astype(bf16)
        )
        bq_c = (np.asarray(b_q, f32)[fsel] * SCL).reshape(2, 128).T
        bk_c = (np.asarray(b_k, f32)[fsel] * SCL).reshape(2, 128).T
        bvt = np.broadcast_to(np.asarray(b_v, f32)[fsel] * SCL, (128, FPC))
        cF_arr = np.concatenate([bq_c, bk_c, bvt], axis=1)
        in_maps.append(
            {
                "cWQ": pack_w(wqT[:, fsel]),
                "cWK": pack_w(wkT[:, fsel]),
                "cWV": pack_w(wvT[:, fsel]),
                "cWO": woc,
                "cMASK": mask4,
                "cF": np.ascontiguousarray(cF_arr.astype(f32)),
                "xq8": xP[("q", g)],
                "xk8": xP[("k", g)],
                "xv8": xP[("v", g)],
            }
        )
    return in_maps


def run(inputs, trace=False):
    from concourse.bass_utils import run_bass_kernel_spmd

    if "nc" not in _CACHE:
        _CACHE["nc"] = _build_nc()
    nc = _CACHE["nc"]
    in_maps = _prep_inputs(
        inputs["query"], inputs["key_"], inputs["value"],
        inputs["w_q"], inputs["b_q"], inputs["w_k"], inputs["b_k"],
        inputs["w_v"], inputs["b_v"], inputs["w_o"], inputs["b_o"],
    )
    res = run_bass_kernel_spmd(
        nc, in_maps, core_ids=list(range(N_CORES)), trace=trace,
    )
    bo = np.asarray(inputs["b_o"], np.float32)
    out = np.empty((B, S, D), np.float32)
    for c in range(N_CORES):
        g, p = c // 4, c % 4
        # RS for wave w scatters q rows [512w + 128p, 512w + 128(p+1))
        core_out = np.asarray(res.results[c]["out"], np.float32)
        for w in range(4):
            out[g, 512 * w + 128 * p : 512 * w + 128 * (p + 1), :] = (
                core_out[128 * w : 128 * (w + 1)] + bo
            )
    return out, res


def kernel(**inputs):
    out, _ = run(inputs, trace=False)
    return out
